# revision 25
# baseline (speedup 1.0000x reference)
"""DeepGT (graph transformer conv, heads=1) on 8 Trainium2 NeuronCores.

v3 strategy (SPMD, one Bass program on 8 cores, per-core data differs):
  - Nodes partitioned into 4 slot-quarters per core (table row count per
    quarter = 8*3200 = 25600 <= int16 range) by a host-side greedy coloring
    that balances each dst node's in-edges across quarters; within each
    quarter nodes are placed round-robin over cores sorted by their max
    per-quarter in-edge count, so per-block static gather widths are tight.
  - Per layer: every core computes K|V for its nodes (bf16), then 4 small
    AllGathers (one per slot-quarter, issued as soon as that quarter's rows
    are final) build 4 global KV tables. The edge gather runs as batched
    dma_gather instructions spread over all 4 SWDGE queues so the four Q7
    core pairs generate descriptors in parallel (per-index DGE time ~9ns is
    the limiting resource; one queue serializes at ~2.5ms/layer).
  - Gather pad slots point at a guaranteed-dummy table row whose K|V is
    exactly zero, so pads contribute exp(0)=1 to the softmax denominator
    (subtracted via a host-side pad count) and nothing to the weighted sum -
    no mask tensor and no mask add on the vector engine.
  - Attention per 128-node block: logits multiply + reduce on DVE, exp with
    fused denominator accumulation on ACT, unnormalized weighted V tree sum
    on DVE, 1/den normalization via ACT scale, skip connection matmul
    accumulated into the transpose PSUM bank (transpose(agg)+Ws^T@hT), next
    layer's K|V matmul fused right after each block's ELU.
  - Residual h kept bf16 in SBUF, transposed layout [feat, node].
"""

import numpy as np
import ml_dtypes
from contextlib import ExitStack

import concourse.bass as bass
import concourse.tile as tile
from concourse import bacc
from concourse import mybir
from concourse.bass_utils import run_bass_kernel_spmd
from concourse.masks import make_identity

P = 128
C = 8   # cores
Q = 4   # slot-quarters (gather index buckets)

F32 = mybir.dt.float32
BF16 = mybir.dt.bfloat16
I16 = mybir.dt.int16

NPC = 12800          # padded slots per core (multiple of 128*Q)
QS = NPC // Q        # slots per quarter = 3200
NB = NPC // P        # blocks per core = 100
QROWS = C * QS       # rows per quarter table = 25600
PAD_ROW = QS - 1     # core 0's last slot per quarter: guaranteed dummy (zero)
# Max summed block width per gather group. Bounded by the SWDGE descriptor
# ring: one dma_gather emits num_idxs/16+1 descriptors per SDMA engine.
# Measured on HW: with single_packet=False, 129 descs/engine works and 193
# crashes (the ring size is runtime-fixed; raising bass's scratch does not
# grow it). Stay at 14 blocks-worth (113 descs) for margin.
WBUDGET = 14


# ----------------------------------------------------------------------------
# Host-side graph planning
# ----------------------------------------------------------------------------

class Plan:
    pass


def _color_quarters(src, dst, deg, order, n_nodes):
    """Assign each node a quarter so each dst's in-edges spread evenly.

    Greedy in degree-desc order + refinement sweeps minimizing the sum of
    per-node max-quarter counts (which directly drives the padded width).
    Quarter capacity is capped 8 below QROWS so every (core, quarter) keeps
    at least one dummy slot (the zero gather-pad row).
    """
    e_by_src = np.argsort(src, kind="stable")
    dst_by_src = dst[e_by_src]
    sstart = np.zeros(n_nodes + 1, np.int64)
    np.cumsum(np.bincount(src, minlength=n_nodes), out=sstart[1:])

    qcap = QROWS - 8
    target = np.ceil(deg / Q).astype(np.int32)
    cnt = np.zeros((n_nodes, Q), np.int32)
    gq = np.zeros(Q, np.int64)
    quarter = np.empty(n_nodes, np.int64)
    for v in order:
        nbrs = dst_by_src[sstart[v]:sstart[v + 1]]
        if nbrs.size:
            over = np.maximum(0, cnt[nbrs] + 1 - target[nbrs, None]).sum(axis=0)
        else:
            over = np.zeros(Q)
        score = over + gq * 1e-5
        score[gq >= qcap] = 1e18
        j = int(np.argmin(score))
        quarter[v] = j
        gq[j] += 1
        if nbrs.size:
            cnt[nbrs, j] += 1

    for _ in range(4):
        moves = 0
        for v in order:
            nbrs = dst_by_src[sstart[v]:sstart[v + 1]]
            if nbrs.size == 0:
                continue
            j0 = quarter[v]
            cn = cnt[nbrs].copy()
            cn[:, j0] -= 1
            base_m = cn.max(axis=1)
            scores = np.empty(Q)
            for j in range(Q):
                scores[j] = np.maximum(base_m, cn[:, j] + 1).sum()
            scores += gq * 1e-5
            full = gq >= qcap
            full[j0] = False
            scores[full] = 1e18
            j = int(np.argmin(scores))
            if j != j0 and scores[j] < scores[j0] - 1e-9:
                quarter[v] = j
                gq[j0] -= 1
                gq[j] += 1
                cnt[nbrs, j0] -= 1
                cnt[nbrs, j] += 1
                moves += 1
        if moves == 0:
            break
    return quarter, cnt


def make_plan(edge_index: np.ndarray, n_nodes: int) -> Plan:
    pl = Plan()
    src = np.asarray(edge_index[0], dtype=np.int64)
    dst = np.asarray(edge_index[1], dtype=np.int64)

    deg = np.bincount(dst, minlength=n_nodes).astype(np.int64)
    order = np.argsort(-deg, kind="stable")

    quarter, cnt = _color_quarters(src, dst, deg, order, n_nodes)

    # placement: per quarter, sort nodes by max-per-quarter count desc,
    # round-robin over cores -> per-core per-block width profiles align
    m = cnt.max(axis=1)
    core_of = np.empty(n_nodes, np.int64)
    slot_of = np.empty(n_nodes, np.int64)
    for j in range(Q):
        nodes = np.where(quarter == j)[0]
        nodes = nodes[np.argsort(-m[nodes], kind="stable")]
        r = np.arange(nodes.size)
        core_of[nodes] = r % C
        slot_of[nodes] = j * QS + r // C

    # bucket-local table row of each node (as a gather source)
    local_row = core_of * QS + (slot_of % QS)  # < QROWS

    # per-edge fields, grouped by dst core
    d_core = core_of[dst]
    d_slot = slot_of[dst]
    e_q = quarter[src]
    e_row = local_row[src]

    # rank of each edge within its (dst, quarter) group
    key = d_slot * Q + e_q + d_core * (NPC * Q)
    e_order = np.argsort(key, kind="stable")
    ks = key[e_order]
    first = np.zeros(len(ks), bool)
    if len(ks):
        first[0] = True
        first[1:] = ks[1:] != ks[:-1]
    grp_start = np.maximum.accumulate(np.where(first, np.arange(len(ks)), 0))
    rank_sorted = np.arange(len(ks)) - grp_start
    rank = np.empty(len(ks), np.int64)
    rank[e_order] = rank_sorted

    b_of = d_slot // P
    p_of = d_slot % P

    cnt4 = np.zeros((C, P, NB, Q), np.int32)
    np.add.at(cnt4, (d_core, p_of, b_of, e_q), 1)
    w_b = cnt4.max(axis=(0, 1, 3)).astype(np.int64)  # static per-block width
    w_b = np.maximum(w_b, 1)
    WMAX = int(w_b.max())

    # gather index array A[c, p, b, q, d]; pads -> the zero dummy row
    A = np.full((C, P, NB, Q, WMAX), PAD_ROW, np.int16)
    A[d_core, p_of, b_of, e_q, rank] = e_row.astype(np.int16)

    # groups of blocks, sum of widths <= WBUDGET (first-fit decreasing;
    # blocks need not be consecutive). Keep groups ordered by their first
    # block so the early-AllGather pipelining still sees quarters complete
    # roughly in order.
    order_w = sorted(range(NB), key=lambda b: -w_b[b])
    bins = []   # (remaining, [blocks])
    for b in order_w:
        for i in range(len(bins)):
            if bins[i][0] >= w_b[b]:
                bins[i][0] -= int(w_b[b])
                bins[i][1].append(b)
                break
        else:
            bins.append([WBUDGET - int(w_b[b]), [b]])
    groups = [sorted(blocks) for _, blocks in bins]
    groups.sort(key=lambda g: g[0])
    pl.groups = groups
    pl.GW = [int(sum(w_b[b] for b in g)) for g in groups]
    pl.GWMAX = max(pl.GW)

    pl.w_b = w_b
    pl.WMAX = WMAX
    pl.SUMW = int(w_b.sum())
    pl.SUMC = 4 * pl.SUMW
    pl.cnt4 = cnt4
    pl.A = A
    pl.core_of = core_of
    pl.slot_of = slot_of
    pl.quarter = quarter
    return pl


def _pack_gidx(pl, c):
    """Per-core int16 index stream: per (group, quarter) a wrapped tile
    [128, 8*GW] where linear idx i=(col*128+p) sits at [i%16, i//16],
    replicated over the 8 partition-groups of 16."""
    parts = []
    for g, blocks in enumerate(pl.groups):
        gw = pl.GW[g]
        for q in range(Q):
            cols = []
            for b in blocks:
                cols.append(pl.A[c, :, b, q, :pl.w_b[b]])  # [P, w_b]
            arr = np.concatenate(cols, axis=1)             # [P, gw]
            lin = arr.T.reshape(-1)                        # i = col*128 + p
            wt = lin.reshape(8 * gw, 16).T                 # [16, 8*gw]
            parts.append(np.tile(wt, (8, 1)))              # [128, 8*gw]
    return np.ascontiguousarray(np.concatenate(parts, axis=1), dtype=np.int16)


def _pack_mask(pl, c):
    """[P, SUMC] f32, block-major (b, q, d): 0 real, -1e4 pads (non-zb)."""
    cols = []
    for b in range(NB):
        cnt = pl.cnt4[c, :, b, :, None]
        d = np.arange(pl.w_b[b])[None, None, :]
        mk = np.where(d < cnt, 0.0, -10000.0)
        cols.append(mk.reshape(P, -1))
    return np.ascontiguousarray(np.concatenate(cols, axis=1), dtype=np.float32)


def _pack_npads(pl, c):
    """[P, NB] f32: number of pad slots per (partition, block)."""
    degs = pl.cnt4[c].sum(axis=2)                 # [P, NB]
    return np.ascontiguousarray(
        (4.0 * pl.w_b[None, :] - degs).astype(np.float32))


# ----------------------------------------------------------------------------
# Bass program
# ----------------------------------------------------------------------------

def build_nc(pl: Plan, L: int, ODIM: int, n_cores: int = C,
             zero_bias: bool = True) -> bass.Bass:
    w_b, groups, GW = pl.w_b, pl.groups, pl.GW
    SUMC, WMAX, GWMAX = pl.SUMC, pl.WMAX, pl.GWMAX
    GIDX_COLS = 32 * sum(GW)

    nc = bacc.Bacc("TRN2", target_bir_lowering=False, debug=False,
                   num_devices=n_cores, num_swdge_queues=4)

    xT_d = nc.dram_tensor("xT", [P, NPC], F32, kind="ExternalInput").ap()
    gidx_d = nc.dram_tensor("gidx", [P, GIDX_COLS], I16, kind="ExternalInput").ap()
    npads_d = nc.dram_tensor("npads", [P, NB], F32, kind="ExternalInput").ap()
    mask_d = nc.dram_tensor("mask", [P, SUMC], F32, kind="ExternalInput").ap()
    linW_d = nc.dram_tensor("linW", [P, P], F32, kind="ExternalInput").ap()
    linb_d = nc.dram_tensor("linb", [P, 1], F32, kind="ExternalInput").ap()
    wkv_d = nc.dram_tensor("wkv", [L, P, 2 * P], BF16, kind="ExternalInput").ap()
    wqs_d = nc.dram_tensor("wqs", [L, P, 2 * P], BF16, kind="ExternalInput").ap()
    bqr_d = nc.dram_tensor("bqr", [L, P, P], F32, kind="ExternalInput").ap()
    bkvr_d = nc.dram_tensor("bkvr", [L, P, 2 * P], F32, kind="ExternalInput").ap()
    bs_d = nc.dram_tensor("bs", [P, max(L, 1)], F32, kind="ExternalInput").ap()
    fcW_d = nc.dram_tensor("fcW", [P, ODIM], BF16, kind="ExternalInput").ap()
    fcbr_d = nc.dram_tensor("fcbr", [P, ODIM], F32, kind="ExternalInput").ap()
    out_d = nc.dram_tensor("out", [NPC, ODIM], F32, kind="ExternalOutput").ap()

    kv_local = nc.dram_tensor("kv_local", [NPC, 2 * P], BF16).ap()
    kv_tab = [[nc.dram_tensor(f"kv_q{j}_p{par}", [QROWS, 2 * P], BF16,
                              addr_space="Shared").ap()
               for j in range(Q)] for par in range(2)]

    with tile.TileContext(nc) as tc, ExitStack() as ctx:
        cp = ctx.enter_context(tc.tile_pool(name="const", bufs=1))
        hp = ctx.enter_context(tc.tile_pool(name="h", bufs=1))
        xp = ctx.enter_context(tc.tile_pool(name="x", bufs=3))
        kvp = ctx.enter_context(tc.tile_pool(name="kv", bufs=2))
        gip = ctx.enter_context(tc.tile_pool(name="gi", bufs=2))
        prp = ctx.enter_context(tc.tile_pool(name="prod", bufs=2))
        sm = ctx.enter_context(tc.tile_pool(name="small", bufs=3))

        # --- constants ---
        npads_sb = cp.tile([P, NB], F32)
        nc.sync.dma_start(out=npads_sb[:], in_=npads_d[:])
        if not zero_bias:
            mask_sb = cp.tile([P, SUMC], F32)
            nc.sync.dma_start(out=mask_sb[:], in_=mask_d[:])
        linW_raw = cp.tile([P, P], F32)
        nc.sync.dma_start(out=linW_raw[:], in_=linW_d[:])
        linW_sb = cp.tile([P, P], F32)
        nc.scalar.activation(linW_sb[:], linW_raw[:],
                             mybir.ActivationFunctionType.Copy)
        linb_sb = cp.tile([P, 1], F32)
        nc.sync.dma_start(out=linb_sb[:], in_=linb_d[:])
        fcW_sb = cp.tile([P, ODIM], BF16)
        nc.sync.dma_start(out=fcW_sb[:], in_=fcW_d[:])
        fcbr_sb = cp.tile([P, ODIM], F32)
        nc.sync.dma_start(out=fcbr_sb[:], in_=fcbr_d[:])
        bs_sb = cp.tile([P, max(L, 1)], F32)
        nc.sync.dma_start(out=bs_sb[:], in_=bs_d[:])
        wkv_sb, wqs_sb, bqr_sb, bkvr_sb = [], [], [], []
        for l in range(L):
            t = cp.tile([P, 2 * P], BF16, tag=f"wkv{l}")
            nc.sync.dma_start(out=t[:], in_=wkv_d[l])
            wkv_sb.append(t)
            t = cp.tile([P, 2 * P], BF16, tag=f"wqs{l}")
            nc.sync.dma_start(out=t[:], in_=wqs_d[l])
            wqs_sb.append(t)
            if not zero_bias:
                t = cp.tile([P, P], F32, tag=f"bqr{l}")
                nc.sync.dma_start(out=t[:], in_=bqr_d[l])
                bqr_sb.append(t)
                t = cp.tile([P, 2 * P], F32, tag=f"bkvr{l}")
                nc.sync.dma_start(out=t[:], in_=bkvr_d[l])
                bkvr_sb.append(t)
        ident = cp.tile([P, P], F32)
        make_identity(nc, ident[:])

        hT = hp.tile([P, NPC], BF16)  # residual stream, [feat, node]

        pp_big = ctx.enter_context(tc.tile_pool(name="ppb", bufs=2, space="PSUM"))
        pp_kv = ctx.enter_context(tc.tile_pool(name="ppkv", bufs=2, space="PSUM"))
        pp_q = ctx.enter_context(tc.tile_pool(name="ppq", bufs=2, space="PSUM"))
        pp_t = ctx.enter_context(tc.tile_pool(name="ppt", bufs=2, space="PSUM"))

        def classifier_block(b):
            blk = slice(b * P, (b + 1) * P)
            po = pp_q.tile([P, P], F32, tag="pq")
            nc.tensor.matmul(po[:, :ODIM], lhsT=hT[:, blk], rhs=fcW_sb[:],
                             start=True, stop=True)
            if zero_bias:
                Lo = po[:, :ODIM]
            else:
                Lot = sm.tile([P, ODIM], F32, tag="Lo")
                nc.vector.tensor_tensor(out=Lot[:], in0=po[:, :ODIM],
                                        in1=fcbr_sb[:],
                                        op=mybir.AluOpType.add)
                Lo = Lot[:]
            mn = sm.tile([P, 1], F32, tag="mn")
            nc.vector.reduce_max(mn[:], Lo, axis=mybir.AxisListType.X,
                                 negate=True)
            eo = sm.tile([P, ODIM], F32, tag="eo")
            dn = sm.tile([P, 1], F32, tag="dn")
            nc.scalar.activation(eo[:], Lo,
                                 mybir.ActivationFunctionType.Exp,
                                 bias=mn[:, 0:1], accum_out=dn[:])
            lnd = sm.tile([P, 1], F32, tag="lnd")
            nc.scalar.activation(lnd[:], dn[:],
                                 mybir.ActivationFunctionType.Ln)
            cc = sm.tile([P, 1], F32, tag="cc")
            nc.vector.tensor_tensor(out=cc[:], in0=mn[:], in1=lnd[:],
                                    op=mybir.AluOpType.subtract)
            oo = sm.tile([P, ODIM], F32, tag="oo")
            nc.scalar.activation(oo[:], Lo,
                                 mybir.ActivationFunctionType.Identity,
                                 bias=cc[:, 0:1])
            nc.sync.dma_start(out=out_d[blk, :], in_=oo[:])

        def kv_block(l, b):
            blk = slice(b * P, (b + 1) * P)
            pkv = pp_kv.tile([P, 2 * P], F32, tag="pkv")
            nc.tensor.matmul(pkv[:], lhsT=hT[:, blk], rhs=wkv_sb[l][:],
                             start=True, stop=True)
            kvs = sm.tile([P, 2 * P], BF16, tag="kvs")
            if zero_bias:
                nc.scalar.activation(kvs[:], pkv[:],
                                     mybir.ActivationFunctionType.Copy)
            else:
                nc.vector.tensor_tensor(out=kvs[:], in0=pkv[:],
                                        in1=bkvr_sb[l][:],
                                        op=mybir.AluOpType.add)
            nc.sync.dma_start(out=kv_local[blk, :], in_=kvs[:])

        def all_gather(l, j):
            nc.gpsimd.collective_compute(
                "AllGather", mybir.AluOpType.bypass,
                replica_groups=[list(range(n_cores))],
                ins=[kv_local[j * QS:(j + 1) * QS, :]],
                outs=[kv_tab[l % 2][j][:]])

        # --- input projection fused with layer-0 K|V: hT = linW.T @ xT,
        # then K|V for the chunk's blocks immediately; AllGather each
        # quarter as soon as its rows are final ---
        CH = 512
        next_ag0 = 0
        for off in range(0, NPC, CH):
            xt = xp.tile([P, CH], F32, tag="xt")
            nc.sync.dma_start(out=xt[:], in_=xT_d[:, off:off + CH])
            pb = pp_big.tile([P, CH], F32, tag="pbig")
            nc.tensor.matmul(pb[:], lhsT=linW_sb[:], rhs=xt[:],
                             start=True, stop=True)
            nc.scalar.activation(hT[:, off:off + CH], pb[:],
                                 mybir.ActivationFunctionType.Identity,
                                 bias=linb_sb[:, 0:1])
            for b in range(off // P, (off + CH) // P):
                kv_block(0, b)
            while (next_ag0 < Q
                   and (off + CH) // P >= (next_ag0 + 1) * (NB // Q)):
                all_gather(0, next_ag0)
                next_ag0 += 1

        # --- layers ---
        mofs_b = np.concatenate([[0], np.cumsum(4 * w_b)]).astype(int)
        for l in range(L):
            par = l % 2
            gofs = 0
            next_ag = 0  # next quarter of layer l+1 to AllGather
            done_b = np.zeros(NB, dtype=bool)
            for g, blocks in enumerate(groups):
                gw = GW[g]
                gi = gip.tile([P, 32 * GWMAX], I16, tag="gi")
                nc.sync.dma_start(out=gi[:, :32 * gw],
                                  in_=gidx_d[:, gofs:gofs + 32 * gw])
                kvt = kvp.tile([P, Q, GWMAX, 2 * P], BF16, tag="kvt")
                for q in range(Q):
                    nc.gpsimd.dma_gather(
                        kvt[:, q, :gw, :], kv_tab[par][q][:],
                        gi[:, 8 * gw * q:8 * gw * (q + 1)],
                        128 * gw, 128 * gw, 2 * P,
                        single_packet=False, queue_num=q)

                ob = 0
                for b in blocks:
                    w = int(w_b[b])
                    s = Q * w
                    blk = slice(b * P, (b + 1) * P)
                    kq = kvt[:, :, ob:ob + w, 0:P]
                    kv = kvt[:, :, ob:ob + w, P:2 * P]

                    pq = pp_q.tile([P, P], F32, tag="pq")
                    nc.tensor.matmul(pq[:], lhsT=hT[:, blk],
                                     rhs=wqs_sb[l][:, 0:P],
                                     start=True, stop=True)
                    qb = sm.tile([P, P], BF16, tag="qb")
                    if zero_bias:
                        nc.scalar.activation(qb[:], pq[:],
                                             mybir.ActivationFunctionType.Copy)
                    else:
                        nc.vector.tensor_tensor(out=qb[:], in0=pq[:],
                                                in1=bqr_sb[l][:],
                                                op=mybir.AluOpType.add)

                    prod = prp.tile([P, Q * WMAX, P], BF16, tag="prod")
                    prod4 = prod[:, :s, :].rearrange(
                        "p (q d) f -> p q d f", q=Q)
                    qbc = qb[:].rearrange("p (a b f) -> p a b f",
                                          a=1, b=1).to_broadcast([P, Q, w, P])
                    nc.vector.tensor_tensor(out=prod4, in0=kq, in1=qbc,
                                            op=mybir.AluOpType.mult)
                    # bf16 reduce output keeps every operand 2-byte, the
                    # requirement for the DVE 2x packed mode (accumulation
                    # is fp32 internally; only the ~+-0.5-magnitude logits
                    # are rounded)
                    Lb = sm.tile([P, Q * WMAX], BF16, tag="Lb")
                    with nc.allow_low_precision("bf16 logits are plenty"):
                        nc.vector.reduce_sum(Lb[:, :s], prod[:, :s, :],
                                             axis=mybir.AxisListType.X)
                    if not zero_bias:
                        mofs = int(mofs_b[b])
                        nc.vector.tensor_tensor(
                            out=Lb[:, :s], in0=Lb[:, :s],
                            in1=mask_sb[:, mofs:mofs + s],
                            op=mybir.AluOpType.add)
                    et = sm.tile([P, Q * WMAX], BF16, tag="et")
                    den = sm.tile([P, 1], F32, tag="den")
                    nc.scalar.activation(et[:, :s], Lb[:, :s],
                                         mybir.ActivationFunctionType.Exp,
                                         accum_out=den[:])
                    den2 = sm.tile([P, 1], F32, tag="den2")
                    if zero_bias:
                        # pads contribute exp(0)=1 each; subtract them
                        nc.vector.tensor_tensor(out=den2[:], in0=den[:],
                                                in1=npads_sb[:, b:b + 1],
                                                op=mybir.AluOpType.subtract)
                        nc.vector.tensor_scalar(out=den2[:], in0=den2[:],
                                                scalar1=1e-30, scalar2=None,
                                                op0=mybir.AluOpType.add)
                    else:
                        nc.vector.tensor_scalar(out=den2[:], in0=den[:],
                                                scalar1=1e-30, scalar2=None,
                                                op0=mybir.AluOpType.add)
                    rden = sm.tile([P, 1], F32, tag="rden")
                    nc.vector.reciprocal(rden[:], den2[:])

                    # unnormalized weighted V sum (pads gather zero rows)
                    # NOTE: the broadcast operand must be in1 - with the
                    # broadcast ap as src0 the DVE runs at 0.92 elem/ns vs
                    # 1.75 elem/ns with the dense tensor first (HW-measured)
                    enc = et[:, :s].rearrange(
                        "p (q d o) -> p q d o", q=Q, o=1).to_broadcast(
                        [P, Q, w, P])
                    nc.vector.tensor_tensor(out=prod4, in0=kv, in1=enc,
                                            op=mybir.AluOpType.mult)
                    dd = s
                    while dd > 1:
                        h2 = (dd + 1) // 2
                        r = dd - h2
                        nc.vector.tensor_tensor(out=prod[:, :r, :],
                                                in0=prod[:, :r, :],
                                                in1=prod[:, h2:dd, :],
                                                op=mybir.AluOpType.add)
                        dd = h2
                    # normalize by 1/den on ACT, then transpose + skip matmul
                    # accumulated in the same PSUM bank
                    sc = sm.tile([P, P], F32, tag="sc")
                    nc.scalar.activation(
                        sc[:], prod[:, 0:1, :].rearrange("p o f -> p (o f)"),
                        mybir.ActivationFunctionType.Copy,
                        scale=rden[:, 0:1])
                    pt = pp_t.tile([P, P], F32, tag="pt")
                    nc.tensor.matmul(pt[:], lhsT=sc[:], rhs=ident[:],
                                     is_transpose=True, start=True, stop=False)
                    nc.tensor.matmul(pt[:], lhsT=wqs_sb[l][:, P:2 * P],
                                     rhs=hT[:, blk], start=False, stop=True)

                    bias = 0.0 if zero_bias else bs_sb[:, l:l + 1]
                    if l < L - 1:
                        es = sm.tile([P, P], F32, tag="es")
                        nc.scalar.activation(es[:], pt[:],
                                             mybir.ActivationFunctionType.Exp,
                                             bias=bias)
                        rs = sm.tile([P, P], F32, tag="rs")
                        nc.scalar.activation(rs[:], pt[:],
                                             mybir.ActivationFunctionType.Relu,
                                             bias=bias)
                        mp = sm.tile([P, P], F32, tag="mp")
                        nc.vector.tensor_scalar(out=mp[:], in0=es[:],
                                                scalar1=1.0, scalar2=0.0,
                                                op0=mybir.AluOpType.subtract,
                                                op1=mybir.AluOpType.min)
                        nc.vector.tensor_tensor(out=hT[:, blk], in0=rs[:],
                                                in1=mp[:],
                                                op=mybir.AluOpType.add)
                        kv_block(l + 1, b)
                    else:
                        nc.scalar.activation(hT[:, blk], pt[:],
                                             mybir.ActivationFunctionType.Identity,
                                             bias=bias)
                        classifier_block(b)
                    ob += w
                gofs += 32 * gw
                # quarter j of layer l+1 is complete once every block of
                # slot range [j*25, (j+1)*25) has run kv_block(l+1, .)
                if l < L - 1:
                    done_b[blocks] = True
                    BQ = NB // Q
                    while (next_ag < Q
                           and done_b[next_ag * BQ:(next_ag + 1) * BQ].all()):
                        all_gather(l + 1, next_ag)
                        next_ag += 1

    nc.compile()
    return nc


# ----------------------------------------------------------------------------
# Host-side input packing
# ----------------------------------------------------------------------------

def make_in_maps(pl: Plan, x, lin_W, lin_b, Wq, bq, Wk, bk, Wv, bv, Ws, bs,
                 fc_W, fc_b, n_cores: int = C):
    L = Wq.shape[0]
    HD = Wq.shape[1]
    ODIM = fc_W.shape[1]
    scale = np.float32(1.0 / np.sqrt(HD))

    wkv = np.concatenate([Wk, Wv], axis=2).astype(ml_dtypes.bfloat16)
    wqs = np.concatenate([Wq * scale, Ws], axis=2).astype(ml_dtypes.bfloat16)
    bqr = np.broadcast_to((bq * scale)[:, None, :], (L, P, HD)).astype(np.float32)
    bkv = np.concatenate([bk, bv], axis=1)
    bkvr = np.broadcast_to(bkv[:, None, :], (L, P, 2 * HD)).astype(np.float32)
    bs_cols = np.ascontiguousarray(bs.T.astype(np.float32))
    fcbr = np.broadcast_to(fc_b[None, :], (P, ODIM)).astype(np.float32)
    lin_bc = np.ascontiguousarray(lin_b.astype(np.float32)[:, None])

    shared = {
        "linW": np.ascontiguousarray(lin_W.astype(np.float32)),
        "linb": lin_bc,
        "wkv": np.ascontiguousarray(wkv),
        "wqs": np.ascontiguousarray(wqs),
        "bqr": np.ascontiguousarray(bqr),
        "bkvr": np.ascontiguousarray(bkvr),
        "bs": bs_cols,
        "fcW": np.ascontiguousarray(fc_W.astype(ml_dtypes.bfloat16)),
        "fcbr": np.ascontiguousarray(fcbr),
    }

    in_maps = []
    for c in range(n_cores):
        xT = np.zeros((P, NPC), dtype=np.float32)
        sel = pl.core_of == c
        xT[:, pl.slot_of[sel]] = x[sel].T.astype(np.float32)
        mdict = dict(shared)
        mdict["xT"] = xT
        mdict["gidx"] = _pack_gidx(pl, c)
        mdict["npads"] = _pack_npads(pl, c)
        mdict["mask"] = _pack_mask(pl, c)
        in_maps.append(mdict)
    return in_maps


def unpack_out(pl: Plan, results, n_nodes: int, ODIM: int):
    out = np.empty((n_nodes, ODIM), dtype=np.float32)
    for c in range(C):
        o = results[c]["out"]
        sel = pl.core_of == c
        out[sel] = o[pl.slot_of[sel]]
    return out


# ----------------------------------------------------------------------------
# Entry point
# ----------------------------------------------------------------------------

def kernel(**inputs) -> np.ndarray:
    x = np.asarray(inputs["x"], dtype=np.float32)
    edge_index = np.asarray(inputs["edge_index"], dtype=np.int32)
    args = {k: np.asarray(v) for k, v in inputs.items()
            if k not in ("x", "edge_index")}

    n_nodes = x.shape[0]
    L = args["Wq"].shape[0]
    ODIM = args["fc_W"].shape[1]

    pl = make_plan(edge_index, n_nodes)
    zb = all(not np.any(args[k])
             for k in ("lin_b", "bq", "bk", "bv", "bs", "fc_b"))
    nc = build_nc(pl, L, ODIM, zero_bias=zb)
    in_maps = make_in_maps(pl, x, args["lin_W"], args["lin_b"],
                           args["Wq"], args["bq"], args["Wk"], args["bk"],
                           args["Wv"], args["bv"], args["Ws"], args["bs"],
                           args["fc_W"], args["fc_b"])
    res = run_bass_kernel_spmd(nc, in_maps, list(range(C)))
    return unpack_out(pl, res.results, n_nodes, ODIM)
